# revision 5
# baseline (speedup 1.0000x reference)
"""Trainium2 Bass kernel for CropSplit (SipMask-style crop + quadrant split).

Reference computation, per output pixel (y, x, n):
    inside = point (x, y) lies in box rois[n] = (x1, y1, x2, y2)
    cell   = which of the 2x2 ROI sub-cells the pixel falls in
    out[y, x, n] = inside ? data[cell, y, x, n] : 0

Strategy (v13 — bf16, column-major fully-packed single pass):
  - bf16 end-to-end (gate is rel_err < 2e-2; bf16 lands ~3e-3): halves
    HBM traffic, and tensor_tensor bf16 hits the 2x_1p DVE perf mode.
  - Shard along W across the 8 cores (25 columns each).  Flatten the
    core's plane COLUMN-major: f = w*H + h, pack f = 40p + r
    (125 partitions x 40 runs, padded to 128).  Each partition then
    covers exactly ONE column w = p//5 (rows 40*(p%5) .. +39), so the
    x-masks are constant per partition: a tiny resident [128, N] pred
    broadcast over runs (fast 1:rb pred:data ratio) instead of a
    1 MB per-element tensor (v8).  y-mask stays per-element u8; the
    combined inside-mask (bf16, for the 2x TT multiply) folds both
    crop conditions into one op.
  - Per run-block, 3 DVE ops:
      cp(dall[0:2], hye, dall[2:4])   y-blend   (2*FD @ ~1.08 cyc/el)
      cp(dall[0],  hxp, dall[1])      x-blend   (FD, broadcast pred)
      dall[0] *= nin                  zero outside (FD @ 2x)
  - Each (c, p) data source block is 40*200 contiguous bf16 = 16 KB.
    Data loads go as one 2D [128, rb*N] DMA per channel (4D APs with
    the big c-stride mid-pattern unbalance the 16 SDMA engines).
  - Queues: Sync = data; Scalar = masks + stores (issue overlap).
    Mask loads prefetch ONE block ahead so the per-block store's
    semaphore wait (scalar-queue head-of-line) never delays the next
    block's mask availability.  Block profile ramps up from 3 runs
    (fast pipeline start) and ends with 4 (short drain tail).
"""

import numpy as np
import ml_dtypes

BF16 = ml_dtypes.bfloat16

C = 2
CC = C * C
H = W = N = 200
NCORES = 8
WS = W // NCORES  # 25 columns per core
F = H * WS  # 5000 flattened (w, h) columns per core
P = 128  # partitions (F padded to P * R with zero columns)
R = 40  # runs per partition
FP = P * R  # 5120 padded columns
PV = F // R  # 125 real partitions

RB_BLOCKS = [(0, 3), (3, 7), (10, 9), (19, 9), (28, 8), (36, 4)]
DATA_BUFS = 8

_cache: dict = {}

# partition/run -> (w_local, h); valid for p < PV
_P_IDX = np.arange(PV)
_P_W = _P_IDX // 5  # column per partition
_PR_H = (40 * (_P_IDX % 5))[:, None] + np.arange(R)[None, :]  # [PV, R]


def _build_module():
    import concourse.bacc as bacc
    import concourse.mybir as mybir
    from concourse.tile import TileContext

    bf16 = mybir.dt.bfloat16
    u8 = mybir.dt.uint8
    mult = mybir.AluOpType.mult

    nc = bacc.Bacc(trn_type="TRN2", debug=False, num_devices=NCORES)
    data = nc.dram_tensor("data", [CC, FP, N], bf16, kind="ExternalInput")
    hxp = nc.dram_tensor("hxp", [128, N], u8, kind="ExternalInput")
    hye = nc.dram_tensor("hye", [128, R, N], u8, kind="ExternalInput")
    nin = nc.dram_tensor("nin", [128, R, N], bf16, kind="ExternalInput")
    out = nc.dram_tensor("out", [FP, N], bf16, kind="ExternalOutput")

    data_r = data.rearrange("c (p r) n -> p c r n", p=P)
    out_r = out.rearrange("(p r) n -> p r n", p=P)

    with TileContext(nc) as tc:
        with (
            tc.tile_pool(name="mpool", bufs=4) as mpool,
            tc.tile_pool(name="spool", bufs=1) as spool,
            tc.tile_pool(name="dpool", bufs=DATA_BUFS) as dpool,
        ):
            hxt = spool.tile([128, N], u8, tag="hxp")
            nc.scalar.dma_start(hxt[:], hxp[:])

            def load_masks(r0, rb):
                sl = slice(r0, r0 + rb)
                hyt = mpool.tile([128, rb, N], u8, tag="hye")
                nc.scalar.dma_start(hyt[:], hye[:, sl, :])
                nint = mpool.tile([128, rb, N], bf16, tag="nin")
                nc.scalar.dma_start(nint[:], nin[:, sl, :])
                return hyt, nint

            # masks prefetch one block ahead so the per-block store's
            # semaphore wait (scalar-queue head-of-line) never delays
            # the next block's mask availability
            mq = [load_masks(*RB_BLOCKS[0])]
            for bi, (r0, rb) in enumerate(RB_BLOCKS):
                sl = slice(r0, r0 + rb)
                dall = dpool.tile([128, CC, rb, N], bf16, tag="dall")
                for c in range(CC):
                    nc.sync.dma_start(dall[:, c], data_r[:, c, sl, :])
                if bi + 1 < len(RB_BLOCKS):
                    mq.append(load_masks(*RB_BLOCKS[bi + 1]))
                hyt, nint = mq[bi]

                nc.vector.copy_predicated(
                    dall[:, 0:2],
                    hyt[:, None, :, :].broadcast_to((128, 2, rb, N)),
                    dall[:, 2:4],
                )
                nc.vector.copy_predicated(
                    dall[:, 0],
                    hxt[:, None, :].broadcast_to((128, rb, N)),
                    dall[:, 1],
                )
                nc.vector.tensor_tensor(
                    dall[:, 0], dall[:, 0], nint[:], mult
                )
                nc.scalar.dma_start(out_r[:, sl, :], dall[:, 0])
    nc.finalize()
    return nc


def _get_module():
    if "nc" not in _cache:
        _cache["nc"] = _build_module()
    return _cache["nc"]


def _host_masks(rois):
    """Masks in f32 arithmetic bit-identical to the reference."""
    r = np.asarray(rois, dtype=np.float32)
    x1, y1, x2, y2 = r[:, 0], r[:, 1], r[:, 2], r[:, 3]
    two = np.float32(2.0)
    one = np.float32(1.0)

    xs = np.arange(W, dtype=np.float32)[:, None]  # (W, 1)
    cw = np.maximum(x2 - x1, one)[None, :]  # (1, N)
    fx = np.floor(two * (xs - x1[None, :]) / cw)
    hx = (fx >= 1.0).astype(np.uint8)  # clip(floor, 0, 1) == 1
    inx = (xs >= x1[None, :]) & (xs <= x2[None, :])

    ys = np.arange(H, dtype=np.float32)[:, None]  # (H, 1)
    ch = np.maximum(y2 - y1, one)[None, :]
    fy = np.floor(two * (ys - y1[None, :]) / ch)
    hy = (fy >= 1.0).astype(np.uint8)
    iny = (ys >= y1[None, :]) & (ys <= y2[None, :])

    return hx, inx, hy, iny


def _run(data, rois, trace=False):
    from concourse.bass_utils import run_bass_kernel_spmd

    data = np.asarray(data, dtype=np.float32).astype(BF16)
    hx, inx, hy, iny = _host_masks(rois)

    in_maps = []
    for i in range(NCORES):
        sl = slice(i * WS, (i + 1) * WS)
        wg = _P_W + i * WS  # global column per partition [PV]
        # x-mask: constant per partition
        hxq = np.zeros((P, N), dtype=np.uint8)
        hxq[:PV] = hx[wg]
        # y-mask and combined inside-mask: per (p, r)
        hyq = np.zeros((P, R, N), dtype=np.uint8)
        hyq[:PV] = hy[_PR_H]
        ninq = np.zeros((P, R, N), dtype=BF16)
        ninq[:PV] = (inx[wg][:, None, :] & iny[_PR_H]).astype(BF16)
        # column-major data: [CC, WS, H, N] flattened to [CC, F, N]
        dpad = np.zeros((CC, FP, N), dtype=BF16)
        dpad[:, :F] = (
            data[:, :, sl, :].transpose(0, 2, 1, 3).reshape(CC, F, N)
        )
        in_maps.append(
            {"data": dpad, "hxp": hxq, "hye": hyq, "nin": ninq}
        )

    nc = _get_module()
    last_err = None
    for _attempt in range(2):
        try:
            res = run_bass_kernel_spmd(
                nc, in_maps, core_ids=list(range(NCORES)), trace=trace
            )
            break
        except Exception as e:  # transient NRT device errors: retry once
            last_err = e
    else:
        raise last_err
    full = np.concatenate(
        [
            r["out"][:F].reshape(WS, H, N).transpose(1, 0, 2)
            for r in res.results
        ],
        axis=1,
    )
    return np.asarray(full).astype(np.float32), res


def kernel(data, rois):
    out, _ = _run(data, rois, trace=False)
    return out


# revision 6
# speedup vs baseline: 1.0972x; 1.0972x over previous
"""Trainium2 Bass kernel for CropSplit (SipMask-style crop + quadrant split).

Reference computation, per output pixel (y, x, n):
    inside = point (x, y) lies in box rois[n] = (x1, y1, x2, y2)
    cell   = which of the 2x2 ROI sub-cells the pixel falls in
    out[y, x, n] = inside ? data[cell, y, x, n] : 0

Strategy (v13 — bf16, column-major fully-packed single pass):
  - bf16 end-to-end (gate is rel_err < 2e-2; bf16 lands ~3e-3): halves
    HBM traffic, and tensor_tensor bf16 hits the 2x_1p DVE perf mode.
  - Shard along W across the 8 cores (25 columns each).  Flatten the
    core's plane COLUMN-major: f = w*H + h, pack f = 40p + r
    (125 partitions x 40 runs, padded to 128).  Each partition then
    covers exactly ONE column w = p//5 (rows 40*(p%5) .. +39), so the
    x-masks are constant per partition: a tiny resident [128, N] pred
    broadcast over runs (fast 1:rb pred:data ratio) instead of a
    1 MB per-element tensor (v8).  y-mask stays per-element u8; the
    combined inside-mask (bf16, for the 2x TT multiply) folds both
    crop conditions into one op.
  - Per run-block, 3 DVE ops:
      cp(dall[0:2], hye, dall[2:4])   y-blend   (2*FD @ ~1.08 cyc/el)
      cp(dall[0],  hxp, dall[1])      x-blend   (FD, broadcast pred)
      dall[0] *= nin                  zero outside (FD @ 2x)
  - Each (c, p) data source block is 40*200 contiguous bf16 = 16 KB.
    Data loads go as one 2D [128, rb*N] DMA per channel (4D APs with
    the big c-stride mid-pattern unbalance the 16 SDMA engines).
  - Queues: Sync = data; Scalar = masks + stores (issue overlap).
    Mask loads prefetch TWO blocks ahead so the per-block store's
    semaphore wait (scalar-queue head-of-line) and supply jitter never
    delay a block's mask availability.  Block profile ramps up from 3 runs
    (fast pipeline start) and ends with 4 (short drain tail).
"""

import numpy as np
import ml_dtypes

BF16 = ml_dtypes.bfloat16

C = 2
CC = C * C
H = W = N = 200
NCORES = 8
WS = W // NCORES  # 25 columns per core
F = H * WS  # 5000 flattened (w, h) columns per core
P = 128  # partitions (F padded to P * R with zero columns)
R = 40  # runs per partition
FP = P * R  # 5120 padded columns
PV = F // R  # 125 real partitions

RB_BLOCKS = [(0, 3), (3, 7), (10, 9), (19, 9), (28, 8), (36, 4)]
DATA_BUFS = 8

_cache: dict = {}

# partition/run -> (w_local, h); valid for p < PV
_P_IDX = np.arange(PV)
_P_W = _P_IDX // 5  # column per partition
_PR_H = (40 * (_P_IDX % 5))[:, None] + np.arange(R)[None, :]  # [PV, R]


def _build_module():
    import concourse.bacc as bacc
    import concourse.mybir as mybir
    from concourse.tile import TileContext

    bf16 = mybir.dt.bfloat16
    u8 = mybir.dt.uint8
    mult = mybir.AluOpType.mult

    nc = bacc.Bacc(trn_type="TRN2", debug=False, num_devices=NCORES)
    data = nc.dram_tensor("data", [CC, FP, N], bf16, kind="ExternalInput")
    hxp = nc.dram_tensor("hxp", [128, N], u8, kind="ExternalInput")
    hye = nc.dram_tensor("hye", [128, R, N], u8, kind="ExternalInput")
    nin = nc.dram_tensor("nin", [128, R, N], bf16, kind="ExternalInput")
    out = nc.dram_tensor("out", [FP, N], bf16, kind="ExternalOutput")

    data_r = data.rearrange("c (p r) n -> p c r n", p=P)
    out_r = out.rearrange("(p r) n -> p r n", p=P)

    with TileContext(nc) as tc:
        with (
            tc.tile_pool(name="mpool", bufs=4) as mpool,
            tc.tile_pool(name="spool", bufs=1) as spool,
            tc.tile_pool(name="dpool", bufs=DATA_BUFS) as dpool,
        ):
            hxt = spool.tile([128, N], u8, tag="hxp")
            nc.scalar.dma_start(hxt[:], hxp[:])

            def load_masks(r0, rb):
                sl = slice(r0, r0 + rb)
                hyt = mpool.tile([128, rb, N], u8, tag="hye")
                nc.scalar.dma_start(hyt[:], hye[:, sl, :])
                nint = mpool.tile([128, rb, N], bf16, tag="nin")
                nc.scalar.dma_start(nint[:], nin[:, sl, :])
                return hyt, nint

            # masks prefetch one block ahead so the per-block store's
            # semaphore wait (scalar-queue head-of-line) never delays
            # the next block's mask availability
            mq = [load_masks(*RB_BLOCKS[0]), load_masks(*RB_BLOCKS[1])]
            for bi, (r0, rb) in enumerate(RB_BLOCKS):
                sl = slice(r0, r0 + rb)
                dall = dpool.tile([128, CC, rb, N], bf16, tag="dall")
                for c in range(CC):
                    nc.sync.dma_start(dall[:, c], data_r[:, c, sl, :])
                if bi + 2 < len(RB_BLOCKS):
                    mq.append(load_masks(*RB_BLOCKS[bi + 2]))
                hyt, nint = mq[bi]

                nc.vector.copy_predicated(
                    dall[:, 0:2],
                    hyt[:, None, :, :].broadcast_to((128, 2, rb, N)),
                    dall[:, 2:4],
                )
                nc.vector.copy_predicated(
                    dall[:, 0],
                    hxt[:, None, :].broadcast_to((128, rb, N)),
                    dall[:, 1],
                )
                nc.vector.tensor_tensor(
                    dall[:, 0], dall[:, 0], nint[:], mult
                )
                nc.scalar.dma_start(out_r[:, sl, :], dall[:, 0])
    nc.finalize()
    return nc


def _get_module():
    if "nc" not in _cache:
        _cache["nc"] = _build_module()
    return _cache["nc"]


def _host_masks(rois):
    """Masks in f32 arithmetic bit-identical to the reference."""
    r = np.asarray(rois, dtype=np.float32)
    x1, y1, x2, y2 = r[:, 0], r[:, 1], r[:, 2], r[:, 3]
    two = np.float32(2.0)
    one = np.float32(1.0)

    xs = np.arange(W, dtype=np.float32)[:, None]  # (W, 1)
    cw = np.maximum(x2 - x1, one)[None, :]  # (1, N)
    fx = np.floor(two * (xs - x1[None, :]) / cw)
    hx = (fx >= 1.0).astype(np.uint8)  # clip(floor, 0, 1) == 1
    inx = (xs >= x1[None, :]) & (xs <= x2[None, :])

    ys = np.arange(H, dtype=np.float32)[:, None]  # (H, 1)
    ch = np.maximum(y2 - y1, one)[None, :]
    fy = np.floor(two * (ys - y1[None, :]) / ch)
    hy = (fy >= 1.0).astype(np.uint8)
    iny = (ys >= y1[None, :]) & (ys <= y2[None, :])

    return hx, inx, hy, iny


def _run(data, rois, trace=False):
    from concourse.bass_utils import run_bass_kernel_spmd

    data = np.asarray(data, dtype=np.float32).astype(BF16)
    hx, inx, hy, iny = _host_masks(rois)

    in_maps = []
    for i in range(NCORES):
        sl = slice(i * WS, (i + 1) * WS)
        wg = _P_W + i * WS  # global column per partition [PV]
        # x-mask: constant per partition
        hxq = np.zeros((P, N), dtype=np.uint8)
        hxq[:PV] = hx[wg]
        # y-mask and combined inside-mask: per (p, r)
        hyq = np.zeros((P, R, N), dtype=np.uint8)
        hyq[:PV] = hy[_PR_H]
        ninq = np.zeros((P, R, N), dtype=BF16)
        ninq[:PV] = (inx[wg][:, None, :] & iny[_PR_H]).astype(BF16)
        # column-major data: [CC, WS, H, N] flattened to [CC, F, N]
        dpad = np.zeros((CC, FP, N), dtype=BF16)
        dpad[:, :F] = (
            data[:, :, sl, :].transpose(0, 2, 1, 3).reshape(CC, F, N)
        )
        in_maps.append(
            {"data": dpad, "hxp": hxq, "hye": hyq, "nin": ninq}
        )

    nc = _get_module()
    last_err = None
    for _attempt in range(2):
        try:
            res = run_bass_kernel_spmd(
                nc, in_maps, core_ids=list(range(NCORES)), trace=trace
            )
            break
        except Exception as e:  # transient NRT device errors: retry once
            last_err = e
    else:
        raise last_err
    full = np.concatenate(
        [
            r["out"][:F].reshape(WS, H, N).transpose(1, 0, 2)
            for r in res.results
        ],
        axis=1,
    )
    return np.asarray(full).astype(np.float32), res


def kernel(data, rois):
    out, _ = _run(data, rois, trace=False)
    return out


# revision 7
# speedup vs baseline: 1.2214x; 1.1133x over previous
"""Trainium2 Bass kernel for CropSplit (SipMask-style crop + quadrant split).

Reference computation, per output pixel (y, x, n):
    inside = point (x, y) lies in box rois[n] = (x1, y1, x2, y2)
    cell   = which of the 2x2 ROI sub-cells the pixel falls in
    out[y, x, n] = inside ? data[cell, y, x, n] : 0

Strategy (v13 — bf16, column-major fully-packed single pass):
  - bf16 end-to-end (gate is rel_err < 2e-2; bf16 lands ~3e-3): halves
    HBM traffic, and tensor_tensor bf16 hits the 2x_1p DVE perf mode.
  - Shard along W across the 8 cores (25 columns each).  Flatten the
    core's plane COLUMN-major: f = w*H + h, pack f = 40p + r
    (125 partitions x 40 runs, padded to 128).  Each partition then
    covers exactly ONE column w = p//5 (rows 40*(p%5) .. +39), so the
    x-masks are constant per partition: a tiny resident [128, N] pred
    broadcast over runs (fast 1:rb pred:data ratio) instead of a
    1 MB per-element tensor (v8).  y-mask stays per-element u8; the
    combined inside-mask (bf16, for the 2x TT multiply) folds both
    crop conditions into one op.
  - Per run-block, 3 DVE ops:
      cp(dall[0:2], hye, dall[2:4])   y-blend   (2*FD @ ~1.08 cyc/el)
      cp(dall[0],  hxp, dall[1])      x-blend   (FD, broadcast pred)
      dall[0] *= nin                  zero outside (FD @ 2x)
  - Each (c, p) data source block is 40*200 contiguous bf16 = 16 KB.
    Data loads go as one 2D [128, rb*N] DMA per channel (4D APs with
    the big c-stride mid-pattern unbalance the 16 SDMA engines).
  - Queues: Sync = data; Scalar = masks + stores (issue overlap).
    The y-pred and inside-mult masks pack into ONE u8 tensor per
    block (bf16 bytes appended; SBUF views them via slice + bitcast),
    halving mask DMA count and semaphore traffic.
    Mask loads prefetch TWO blocks ahead so the per-block store's
    semaphore wait (scalar-queue head-of-line) and supply jitter never
    delay a block's mask availability.  Block profile ramps up from 3 runs
    (fast pipeline start) and ends with 4 (short drain tail).
"""

import numpy as np
import ml_dtypes

BF16 = ml_dtypes.bfloat16

C = 2
CC = C * C
H = W = N = 200
NCORES = 8
WS = W // NCORES  # 25 columns per core
F = H * WS  # 5000 flattened (w, h) columns per core
P = 128  # partitions (F padded to P * R with zero columns)
R = 40  # runs per partition
FP = P * R  # 5120 padded columns
PV = F // R  # 125 real partitions

RB_BLOCKS = [(0, 3), (3, 7), (10, 9), (19, 9), (28, 8), (36, 4)]
DATA_BUFS = 8

_cache: dict = {}

# partition/run -> (w_local, h); valid for p < PV
_P_IDX = np.arange(PV)
_P_W = _P_IDX // 5  # column per partition
_PR_H = (40 * (_P_IDX % 5))[:, None] + np.arange(R)[None, :]  # [PV, R]


def _build_module():
    import concourse.bacc as bacc
    import concourse.mybir as mybir
    from concourse.tile import TileContext

    bf16 = mybir.dt.bfloat16
    u8 = mybir.dt.uint8
    mult = mybir.AluOpType.mult

    nc = bacc.Bacc(trn_type="TRN2", debug=False, num_devices=NCORES)
    data = nc.dram_tensor("data", [CC, FP, N], bf16, kind="ExternalInput")
    hxp = nc.dram_tensor("hxp", [128, N], u8, kind="ExternalInput")
    # packed per-element masks: bytes [0:N] = hye u8 pred,
    # bytes [N:3N] = nin bf16 inside-mult (viewed via bitcast on SBUF)
    hn = nc.dram_tensor("hn", [128, R, 3 * N], u8, kind="ExternalInput")
    out = nc.dram_tensor("out", [FP, N], bf16, kind="ExternalOutput")

    data_r = data.rearrange("c (p r) n -> p c r n", p=P)
    out_r = out.rearrange("(p r) n -> p r n", p=P)

    with TileContext(nc) as tc:
        with (
            tc.tile_pool(name="mpool", bufs=4) as mpool,
            tc.tile_pool(name="spool", bufs=1) as spool,
            tc.tile_pool(name="dpool", bufs=DATA_BUFS) as dpool,
        ):
            hxt = spool.tile([128, N], u8, tag="hxp")
            nc.scalar.dma_start(hxt[:], hxp[:])

            def load_masks(r0, rb):
                sl = slice(r0, r0 + rb)
                hnt = mpool.tile([128, rb, 3 * N], u8, tag="hn")
                nc.scalar.dma_start(hnt[:], hn[:, sl, :])
                hyt = hnt[:, :, 0:N]
                nint = hnt[:, :, N : 3 * N].bitcast(bf16)
                return hyt, nint

            # masks prefetch one block ahead so the per-block store's
            # semaphore wait (scalar-queue head-of-line) never delays
            # the next block's mask availability
            mq = [load_masks(*RB_BLOCKS[0]), load_masks(*RB_BLOCKS[1])]
            for bi, (r0, rb) in enumerate(RB_BLOCKS):
                sl = slice(r0, r0 + rb)
                dall = dpool.tile([128, CC, rb, N], bf16, tag="dall")
                for c in range(CC):
                    nc.sync.dma_start(dall[:, c], data_r[:, c, sl, :])
                if bi + 2 < len(RB_BLOCKS):
                    mq.append(load_masks(*RB_BLOCKS[bi + 2]))
                hyt, nint = mq[bi]

                nc.vector.copy_predicated(
                    dall[:, 0:2],
                    hyt[:, None, :, :].broadcast_to((128, 2, rb, N)),
                    dall[:, 2:4],
                )
                nc.vector.copy_predicated(
                    dall[:, 0],
                    hxt[:, None, :].broadcast_to((128, rb, N)),
                    dall[:, 1],
                )
                nc.vector.tensor_tensor(
                    dall[:, 0], dall[:, 0], nint[:], mult
                )
                nc.scalar.dma_start(out_r[:, sl, :], dall[:, 0])
    nc.finalize()
    return nc


def _get_module():
    if "nc" not in _cache:
        _cache["nc"] = _build_module()
    return _cache["nc"]


def _host_masks(rois):
    """Masks in f32 arithmetic bit-identical to the reference."""
    r = np.asarray(rois, dtype=np.float32)
    x1, y1, x2, y2 = r[:, 0], r[:, 1], r[:, 2], r[:, 3]
    two = np.float32(2.0)
    one = np.float32(1.0)

    xs = np.arange(W, dtype=np.float32)[:, None]  # (W, 1)
    cw = np.maximum(x2 - x1, one)[None, :]  # (1, N)
    fx = np.floor(two * (xs - x1[None, :]) / cw)
    hx = (fx >= 1.0).astype(np.uint8)  # clip(floor, 0, 1) == 1
    inx = (xs >= x1[None, :]) & (xs <= x2[None, :])

    ys = np.arange(H, dtype=np.float32)[:, None]  # (H, 1)
    ch = np.maximum(y2 - y1, one)[None, :]
    fy = np.floor(two * (ys - y1[None, :]) / ch)
    hy = (fy >= 1.0).astype(np.uint8)
    iny = (ys >= y1[None, :]) & (ys <= y2[None, :])

    return hx, inx, hy, iny


def _run(data, rois, trace=False):
    from concourse.bass_utils import run_bass_kernel_spmd

    data = np.asarray(data, dtype=np.float32).astype(BF16)
    hx, inx, hy, iny = _host_masks(rois)

    in_maps = []
    for i in range(NCORES):
        sl = slice(i * WS, (i + 1) * WS)
        wg = _P_W + i * WS  # global column per partition [PV]
        # x-mask: constant per partition
        hxq = np.zeros((P, N), dtype=np.uint8)
        hxq[:PV] = hx[wg]
        # y-mask and combined inside-mask: per (p, r)
        hnq = np.zeros((P, R, 3 * N), dtype=np.uint8)
        hnq[:PV, :, :N] = hy[_PR_H]
        ninq = np.zeros((P, R, N), dtype=BF16)
        ninq[:PV] = (inx[wg][:, None, :] & iny[_PR_H]).astype(BF16)
        hnq[:, :, N:] = ninq.view(np.uint8)
        # column-major data: [CC, WS, H, N] flattened to [CC, F, N]
        dpad = np.zeros((CC, FP, N), dtype=BF16)
        dpad[:, :F] = (
            data[:, :, sl, :].transpose(0, 2, 1, 3).reshape(CC, F, N)
        )
        in_maps.append(
            {"data": dpad, "hxp": hxq, "hn": hnq}
        )

    nc = _get_module()
    last_err = None
    for _attempt in range(2):
        try:
            res = run_bass_kernel_spmd(
                nc, in_maps, core_ids=list(range(NCORES)), trace=trace
            )
            break
        except Exception as e:  # transient NRT device errors: retry once
            last_err = e
    else:
        raise last_err
    full = np.concatenate(
        [
            r["out"][:F].reshape(WS, H, N).transpose(1, 0, 2)
            for r in res.results
        ],
        axis=1,
    )
    return np.asarray(full).astype(np.float32), res


def kernel(data, rois):
    out, _ = _run(data, rois, trace=False)
    return out
